# revision 2
# baseline (speedup 1.0000x reference)
"""GQA attention prefill kernel for Trainium2 (Bass/Tile), 8-way tensor
parallel over heads.

Problem (hardcoded): B=1, S=2048, HID=4096, NH=32, KVH=8, D=128, causal
prefill with per-head RMSNorm on q/k and RoPE, positions = arange(S).

Sharding: core c owns kv-head c and q-heads 4c..4c+3. wq/wo sharded on the
head dim, wk/wv on the kv-head dim; x, rope tables replicated. Each core
computes its 4 heads' contribution through wo; the host sums the 8 partial
outputs.

Host-side prep (part of sharding): x and the weight shards are fed
pre-transposed (contraction dim major) so the device never transposes
activations or weights; the q/k norm weights and the rotate-half sign are
folded into transposed rope tables.

All big matmuls run in float32r (full PE rate at free dim >= 256). The
walrus verifier requires f32r matmul operands to be produced as f32r, so
every tile feeding a matmul is written with dtype float32r by its producer.
Cross-partition reductions (rms-norm sums, softmax denominators) use a
ones[128,128] lhsT, which yields the column sums already broadcast across
all 128 partitions - no degenerate M=1/N=1 matmuls and no separate
broadcast step.
"""

import numpy as np

import concourse.bass as bass
import concourse.mybir as mybir
import concourse.tile as tile
from concourse import bacc
from concourse.masks import make_identity

P = 128
S = 2048
HID = 4096
D = 128
G = 4            # q heads per core
NHT = HID // P   # 32 h-tiles (contraction)
SC = 512         # seq chunk
NSC = S // SC    # 4
NKT = S // P     # 16 k-tiles
EPS = 1e-6
N_CORES = 8

F32 = mybir.dt.float32
F32R = mybir.dt.float32r


def build_program():
    nc = bacc.Bacc("TRN2", target_bir_lowering=False, debug=False)

    xT = nc.dram_tensor("xT", [HID, S], F32R, kind="ExternalInput").ap()
    wqT = nc.dram_tensor("wqT", [HID, G * P], F32R, kind="ExternalInput").ap()
    wkT = nc.dram_tensor("wkT", [HID, P], F32R, kind="ExternalInput").ap()
    wvT = nc.dram_tensor("wvT", [HID, P], F32R, kind="ExternalInput").ap()
    woT = nc.dram_tensor("woT", [G * P, HID], F32R, kind="ExternalInput").ap()
    cosq = nc.dram_tensor("cosq", [D, S], F32, kind="ExternalInput").ap()
    sinq = nc.dram_tensor("sinq", [D, S], F32, kind="ExternalInput").ap()
    cosk = nc.dram_tensor("cosk", [D, S], F32, kind="ExternalInput").ap()
    sink = nc.dram_tensor("sink", [D, S], F32, kind="ExternalInput").ap()
    y = nc.dram_tensor("y", [S, HID], F32, kind="ExternalOutput").ap()

    Sqrt = mybir.ActivationFunctionType.Sqrt
    Exp = mybir.ActivationFunctionType.Exp

    with tile.TileContext(nc) as tc:
        with (
            tc.tile_pool(name="const", bufs=1) as const,
            tc.tile_pool(name="tabs", bufs=1) as tabs,
            tc.tile_pool(name="xw", bufs=3) as xw,
            tc.tile_pool(name="scr", bufs=2) as scr,
            tc.tile_pool(name="qrp", bufs=5) as qrp,
            tc.tile_pool(name="otp", bufs=5) as otp,
            tc.tile_pool(name="ptp", bufs=4) as ptp,
            tc.tile_pool(name="yp", bufs=3) as yp,
            tc.tile_pool(name="ps", bufs=8, space="PSUM") as ps,
        ):
            identity = const.tile([P, P], F32)
            make_identity(nc, identity)
            # gpsimd memset can't write f32r -> build constants in an f32
            # scratch and DVE-copy (rounding convert) into the f32r tiles.
            f32tmp = const.tile([P, SC], F32)
            # ones_full[k, m] == 1: matmul(out, ones_full, rhs) gives column
            # sums of rhs broadcast across all 128 output partitions.
            ones_full = const.tile([P, P], F32R)
            nc.gpsimd.memset(f32tmp, 1.0)
            nc.vector.tensor_copy(ones_full, f32tmp[:, 0:P])
            # causal masks for the 4 diagonal k-tiles of a q chunk:
            # keep (1.0) where q_local >= 128*j + k_local
            masks = []
            for j in range(4):
                mk = const.tile([P, SC], F32R, name=f"mask{j}")
                nc.gpsimd.memset(f32tmp, 1.0)
                nc.gpsimd.affine_select(
                    f32tmp, f32tmp, pattern=[[1, SC]],
                    compare_op=mybir.AluOpType.is_ge,
                    fill=0.0, base=-P * j, channel_multiplier=-1,
                )
                nc.vector.tensor_copy(mk, f32tmp)
                masks.append(mk)

            bias_keps = const.tile([P, 1], F32)
            nc.gpsimd.memset(bias_keps, float(P) * EPS)
            bias_qeps = const.tile([P, 1], F32)
            nc.gpsimd.memset(bias_qeps, EPS)

            KR = const.tile([P, S], F32R)       # roped+scaled K, [d, s]
            Vs = const.tile([P, NKT, P], F32R)  # V, [s-in-tile, k-tile, d]

            woT_sb = const.tile([P, G, HID], F32R)
            for mt in range(G):
                nc.sync.dma_start(woT_sb[:, mt, :], woT[mt * P:(mt + 1) * P, :])

            for sc in range(NSC):
                q0 = sc * SC
                cq = tabs.tile([P, SC], F32, tag="cosq")
                nc.sync.dma_start(cq, cosq[:, q0:q0 + SC])
                sq_ = tabs.tile([P, SC], F32, tag="sinq")
                nc.sync.dma_start(sq_, sinq[:, q0:q0 + SC])
                ck = tabs.tile([P, SC], F32, tag="cosk")
                nc.sync.dma_start(ck, cosk[:, q0:q0 + SC])
                sk = tabs.tile([P, SC], F32, tag="sink")
                nc.sync.dma_start(sk, sink[:, q0:q0 + SC])

                # ---- projections: accumulate over 32 h-tiles ----
                qps = [ps.tile([P, SC], F32, tag="ps", name=f"qps{i}")
                       for i in range(G)]
                kps = ps.tile([P, SC], F32, tag="ps")
                vps = ps.tile([P, SC], F32, tag="ps")
                for ht in range(NHT):
                    h0 = ht * P
                    xt = xw.tile([P, SC], F32R, tag="xt")
                    nc.sync.dma_start(xt, xT[h0:h0 + P, q0:q0 + SC])
                    wqt = xw.tile([P, G * P], F32R, tag="wqt")
                    nc.sync.dma_start(wqt, wqT[h0:h0 + P, :])
                    wkt = xw.tile([P, P], F32R, tag="wkt")
                    nc.sync.dma_start(wkt, wkT[h0:h0 + P, :])
                    wvt = xw.tile([P, P], F32R, tag="wvt")
                    nc.sync.dma_start(wvt, wvT[h0:h0 + P, :])
                    st = ht == 0
                    sp = ht == NHT - 1
                    for mt in range(G):
                        nc.tensor.matmul(
                            qps[mt], wqt[:, mt * P:(mt + 1) * P], xt,
                            start=st, stop=sp,
                        )
                    nc.tensor.matmul(kps, wkt, xt, start=st, stop=sp)
                    nc.tensor.matmul(vps, wvt, xt, start=st, stop=sp)

                # ---- K: rope + fold per-k norm scale into KR columns ----
                kraw = scr.tile([P, SC], F32, tag="kraw")
                nc.vector.tensor_copy(kraw, kps)
                sqk = scr.tile([P, SC], F32R, tag="sq")
                nc.vector.tensor_mul(sqk, kraw, kraw)
                ssb = ps.tile([P, SC], F32, tag="ps")
                nc.tensor.matmul(ssb, ones_full, sqk, start=True, stop=True)
                # 1/sqrt(ssq + d*eps) == rsqrt(mean+eps)/sqrt(d):
                # k-norm and softmax 1/sqrt(d) in one factor
                rkf = scr.tile([P, SC], F32, tag="rk")
                nc.scalar.activation(rkf, ssb, Sqrt, bias=bias_keps, scale=1.0)
                nc.vector.reciprocal(rkf, rkf)
                # rope: out = z*cos + rot(z)*sin_eff, rot(z) = [z2; z1]
                # (rotate-half minus sign folded into sin_eff on host)
                krot = scr.tile([P, SC], F32, tag="krot")
                nc.sync.dma_start(krot[0:64], kraw[64:128])
                nc.sync.dma_start(krot[64:128], kraw[0:64])
                t1 = scr.tile([P, SC], F32, tag="t1")
                nc.vector.tensor_mul(t1, krot, sk)
                kpre = scr.tile([P, SC], F32, tag="kpre")
                nc.vector.tensor_mul(kpre, kraw, ck)
                nc.vector.tensor_add(kpre, kpre, t1)
                nc.vector.tensor_mul(KR[:, q0:q0 + SC], kpre, rkf)

                # ---- V: transpose [d, s] -> [s, d] tiles ----
                vtmp = scr.tile([P, SC], F32, tag="vtmp")
                nc.vector.tensor_copy(vtmp, vps)
                for j in range(SC // P):
                    tp = ps.tile([P, P], F32, tag="ps")
                    nc.tensor.transpose(tp, vtmp[:, j * P:(j + 1) * P], identity)
                    nc.vector.tensor_copy(Vs[:, sc * 4 + j, :], tp)

                # ---- Q per head: rope + norm scale ----
                qrs = []
                for h in range(G):
                    qraw = scr.tile([P, SC], F32, tag="qraw")
                    nc.vector.tensor_copy(qraw, qps[h])
                    sqq = scr.tile([P, SC], F32R, tag="sq")
                    nc.vector.tensor_mul(sqq, qraw, qraw)
                    ssbq = ps.tile([P, SC], F32, tag="ps")
                    nc.tensor.matmul(ssbq, ones_full, sqq,
                                     start=True, stop=True)
                    rqf = scr.tile([P, SC], F32, tag="rk")
                    nc.scalar.activation(rqf, ssbq, Sqrt,
                                         bias=bias_qeps, scale=1.0 / P)
                    nc.vector.reciprocal(rqf, rqf)
                    qrot = scr.tile([P, SC], F32, tag="krot")
                    nc.sync.dma_start(qrot[0:64], qraw[64:128])
                    nc.sync.dma_start(qrot[64:128], qraw[0:64])
                    t1b = scr.tile([P, SC], F32, tag="t1")
                    nc.vector.tensor_mul(t1b, qrot, sq_)
                    qpre = scr.tile([P, SC], F32, tag="kpre")
                    nc.vector.tensor_mul(qpre, qraw, cq)
                    nc.vector.tensor_add(qpre, qpre, t1b)
                    qr = qrp.tile([P, SC], F32R, tag="qr")
                    nc.vector.tensor_mul(qr, qpre, rqf)
                    qrs.append(qr)

                # ---- attention for this q chunk ----
                ots = []
                for h in range(G):
                    avp = ps.tile([P, SC], F32, tag="ps")
                    dnp = ps.tile([P, SC], F32, tag="ps")
                    nkt = (sc + 1) * 4
                    for kt in range(nkt):
                        ptps = ps.tile([P, SC], F32, tag="ps")
                        nc.tensor.matmul(
                            ptps, KR[:, kt * P:(kt + 1) * P], qrs[h],
                            start=True, stop=True,
                        )
                        pt = ptp.tile([P, SC], F32R, tag="pt")
                        nc.scalar.activation(pt, ptps, Exp, bias=0.0,
                                             scale=1.0)
                        if kt >= sc * 4:
                            nc.vector.tensor_mul(pt, pt, masks[kt - sc * 4])
                        nc.tensor.matmul(dnp, ones_full, pt,
                                         start=(kt == 0), stop=(kt == nkt - 1))
                        nc.tensor.matmul(avp, Vs[:, kt, :], pt,
                                         start=(kt == 0), stop=(kt == nkt - 1))
                    rcp = scr.tile([P, SC], F32, tag="rcp")
                    nc.vector.reciprocal(rcp, dnp)
                    av_sb = scr.tile([P, SC], F32, tag="av")
                    nc.vector.tensor_copy(av_sb, avp)
                    ot = otp.tile([P, SC], F32R, tag="ot")
                    nc.vector.tensor_mul(ot, av_sb, rcp)
                    ots.append(ot)

                # ---- output projection for this q chunk ----
                for stl in range(SC // P):
                    srow = q0 + stl * P
                    for grp in range(2):
                        yps_l = [ps.tile([P, SC], F32, tag="ps", name=f"yps{j}")
                                 for j in range(4)]
                        for h in range(G):
                            lhs = ots[h][:, stl * P:(stl + 1) * P]
                            for j in range(4):
                                hc = grp * 4 + j
                                nc.tensor.matmul(
                                    yps_l[j], lhs,
                                    woT_sb[:, h, hc * SC:(hc + 1) * SC],
                                    start=(h == 0), stop=(h == G - 1),
                                )
                        for j in range(4):
                            hc = grp * 4 + j
                            ys = yp.tile([P, SC], F32, tag="ys")
                            nc.scalar.copy(ys, yps_l[j])
                            nc.sync.dma_start(
                                y[srow:srow + P, hc * SC:(hc + 1) * SC], ys)

    nc.finalize()
    return nc


def shard_inputs(x, wq, wk, wv, wo, q_norm_w, k_norm_w, cos_table, sin_table,
                 positions, **_ignored):
    """Host-side sharding: returns the list of 8 per-core input maps."""
    x = np.asarray(x, np.float32)
    pos = np.asarray(positions).astype(np.int64)
    cos_sel = np.asarray(cos_table, np.float32)[pos]   # [S, D]
    sin_sel = np.asarray(sin_table, np.float32)[pos]
    qw = np.asarray(q_norm_w, np.float32)
    kw = np.asarray(k_norm_w, np.float32)
    # fold norm weights into the transposed rope tables:
    # w * rope(q') == q'*(w*cos) + rot(q')*(w*sin)
    # also fold rotate-half's minus sign into sin rows 0..63:
    # rope(z) = z*cos + [-z2; z1]*sin = z*cos + [z2; z1]*sin_eff
    sign = np.ones((1, D), np.float32)
    sign[0, :D // 2] = -1.0
    cosq = np.ascontiguousarray((cos_sel * qw).T)      # [D, S]
    sinq = np.ascontiguousarray((sin_sel * qw * sign).T)
    cosk = np.ascontiguousarray((cos_sel * kw).T)
    sink = np.ascontiguousarray((sin_sel * kw * sign).T)
    xTf = np.ascontiguousarray(x.reshape(S, HID).T)    # [HID, S]
    wq = np.asarray(wq, np.float32)
    wk = np.asarray(wk, np.float32)
    wv = np.asarray(wv, np.float32)
    wo = np.asarray(wo, np.float32)

    in_maps = []
    for c in range(N_CORES):
        m = {
            "xT": xTf,
            "wqT": np.ascontiguousarray(wq[c * G * P:(c + 1) * G * P, :].T),
            "wkT": np.ascontiguousarray(wk[c * P:(c + 1) * P, :].T),
            "wvT": np.ascontiguousarray(wv[c * P:(c + 1) * P, :].T),
            "woT": np.ascontiguousarray(wo[:, c * G * P:(c + 1) * G * P].T),
            "cosq": cosq, "sinq": sinq, "cosk": cosk, "sink": sink,
        }
        in_maps.append(m)
    return in_maps


_NC = None


def _get_nc():
    global _NC
    if _NC is None:
        _NC = build_program()
    return _NC


def run_on_device(in_maps, trace=False, **kw):
    from concourse.bass_utils import run_bass_kernel_spmd
    nc = _get_nc()
    return run_bass_kernel_spmd(nc, in_maps, list(range(N_CORES)),
                                trace=trace, **kw)


def kernel(**inputs):
    in_maps = shard_inputs(**inputs)
    res = run_on_device(in_maps).results
    y = np.zeros((S, HID), np.float32)
    for c in range(N_CORES):
        y += res[c]["y"]
    return y.reshape(1, S, HID)



# revision 3
# speedup vs baseline: 1.4700x; 1.4700x over previous
"""GQA attention prefill kernel for Trainium2 (Bass/Tile), 8-way tensor
parallel over heads.

Problem (hardcoded): B=1, S=2048, HID=4096, NH=32, KVH=8, D=128, causal
prefill with per-head RMSNorm on q/k and RoPE, positions = arange(S).

Sharding: core c owns kv-head c and q-heads 4c..4c+3. wq/wo sharded on the
head dim, wk/wv on the kv-head dim; x, rope tables replicated. Each core
computes its 4 heads' contribution through wo; the host sums the 8 partial
outputs.

v2 design (vs the f32r baseline):
- Full bf16 datapath (CPU-simulated rel err 3.4e-3 vs the 2e-2 gate).
  Matmuls accumulate in f32 PSUM; bf16 moving operands run at the same
  1 cycle/row PE rate as f32r but halve DMA/SBUF and double DVE rate.
- All weights resident in SBUF (10.5 MB bf16), loaded once. The f32
  baseline re-streamed wq/wk/wv every chunk and was DMA-bound during
  projections (~240 GB/s sustained, PE idle on LDWEIGHTS waits).
- Two phases: (A) QKV projections for all 4 seq chunks, with each chunk's
  PSUM drains + rope/rms-norm emitted AFTER the next chunk's projection
  matmuls so the DVE/Act work hides under PE work; (B) attention +
  output projection per chunk.
- K-side rms-norm factor (which also carries softmax 1/sqrt(d)) applied
  as the per-partition `scale` operand of the Exp activation, computed
  directly in transposed [k,1] layout via tiny matmuls against a ones
  column. Kills the broadcast sqrt/reciprocal/multiply on the K path.
- Reciprocals via the ~5x faster reciprocal_approx_fast custom DVE op
  (the baseline burned 120us in exact InstReciprocal).
- Output PSUM drains alternate between Scalar and Vector engines.
"""

import numpy as np

import concourse.bass as bass
import concourse.mybir as mybir
import concourse.tile as tile
from concourse import bacc

P = 128
S = 2048
HID = 4096
D = 128
G = 4            # q heads per core
NHT = HID // P   # 32 h-tiles (contraction)
SC = 512         # seq chunk
NSC = S // SC    # 4
NKT = S // P     # 16 k-tiles
EPS = 1e-6
N_CORES = 8

F32 = mybir.dt.float32
BF16 = mybir.dt.bfloat16

Sqrt = mybir.ActivationFunctionType.Sqrt
Exp = mybir.ActivationFunctionType.Exp


def build_program():
    nc = bacc.Bacc("TRN2", target_bir_lowering=False, debug=False)

    xT = nc.dram_tensor("xT", [HID, S], BF16, kind="ExternalInput").ap()
    wqT = nc.dram_tensor("wqT", [HID, G * P], BF16, kind="ExternalInput").ap()
    wkT = nc.dram_tensor("wkT", [HID, P], BF16, kind="ExternalInput").ap()
    wvT = nc.dram_tensor("wvT", [HID, P], BF16, kind="ExternalInput").ap()
    woT = nc.dram_tensor("woT", [G * P, HID], BF16, kind="ExternalInput").ap()
    cosq = nc.dram_tensor("cosq", [D, S], BF16, kind="ExternalInput").ap()
    sinq = nc.dram_tensor("sinq", [D, S], BF16, kind="ExternalInput").ap()
    cosk = nc.dram_tensor("cosk", [D, S], BF16, kind="ExternalInput").ap()
    sink = nc.dram_tensor("sink", [D, S], BF16, kind="ExternalInput").ap()
    y = nc.dram_tensor("y", [S, HID], BF16, kind="ExternalOutput").ap()

    with tile.TileContext(nc) as tc:
        with (
            tc.tile_pool(name="const", bufs=1) as const,
            tc.tile_pool(name="xw", bufs=6) as xw,
            tc.tile_pool(name="scr", bufs=2) as scr,
            tc.tile_pool(name="ptp", bufs=3) as ptp,
            tc.tile_pool(name="otp", bufs=5) as otp,
            tc.tile_pool(name="ysp", bufs=4) as ysp,
            tc.tile_pool(name="ps", bufs=1, space="PSUM") as ps,
        ):
            # ---- constants ----
            f32tmp = const.tile([P, SC], F32)
            identity = const.tile([P, P], BF16)
            nc.gpsimd.memset(f32tmp[:, 0:P], 0.0)
            nc.gpsimd.affine_select(
                f32tmp[:, 0:P], f32tmp[:, 0:P],
                compare_op=mybir.AluOpType.not_equal, fill=1.0,
                base=0, pattern=[[-1, P]], channel_multiplier=1,
            )
            nc.vector.tensor_copy(identity, f32tmp[:, 0:P])
            ones_bf = const.tile([P, P], BF16)
            nc.gpsimd.memset(f32tmp[:, 0:P], 1.0)
            nc.vector.tensor_copy(ones_bf, f32tmp[:, 0:P])
            # causal masks for the 4 diagonal k-tiles of a q chunk:
            # keep (1.0) where q_local >= 128*j + k_local
            masks = []
            for j in range(4):
                mk = const.tile([P, SC], BF16, name=f"mask{j}")
                nc.gpsimd.memset(f32tmp, 1.0)
                nc.gpsimd.affine_select(
                    f32tmp, f32tmp, pattern=[[1, SC]],
                    compare_op=mybir.AluOpType.is_ge,
                    fill=0.0, base=-P * j, channel_multiplier=-1,
                )
                nc.vector.tensor_copy(mk, f32tmp)
                masks.append(mk)

            bias_keps = const.tile([P, 1], F32)
            nc.gpsimd.memset(bias_keps, float(P) * EPS)
            bias_qeps = const.tile([P, 1], F32)
            nc.gpsimd.memset(bias_qeps, EPS)

            # ---- persistent tensors ----
            KR = const.tile([P, S], BF16)        # roped K (no norm factor)
            Vs = const.tile([P, NKT, P], BF16)   # V, [s-in-tile, k-tile, d]
            qr_all = const.tile([P, G, S], BF16)  # roped+normed Q per head
            rkfT = const.tile([P, NKT], F32)     # k-norm factor, [k-in-tile, k-tile]

            # ---- resident weights (loaded once; per-ht DMAs so chunk-0
            # projections can start as soon as their slice lands) ----
            wq_sb = const.tile([P, NHT, G * P], BF16)
            wk_sb = const.tile([P, NHT, P], BF16)
            wv_sb = const.tile([P, NHT, P], BF16)
            for ht in range(NHT):
                nc.sync.dma_start(wq_sb[:, ht, :], wqT[ht * P:(ht + 1) * P, :])
                nc.sync.dma_start(wk_sb[:, ht, :], wkT[ht * P:(ht + 1) * P, :])
                nc.sync.dma_start(wv_sb[:, ht, :], wvT[ht * P:(ht + 1) * P, :])
            cq_sb = const.tile([P, S], BF16)
            sq_sb = const.tile([P, S], BF16)
            ck_sb = const.tile([P, S], BF16)
            sk_sb = const.tile([P, S], BF16)
            nc.sync.dma_start(ck_sb, cosk)
            nc.sync.dma_start(sk_sb, sink)
            nc.sync.dma_start(cq_sb, cosq)
            nc.sync.dma_start(sq_sb, sinq)
            wo_sb = const.tile([P, G, HID], BF16)
            for mt in range(G):
                nc.sync.dma_start(wo_sb[:, mt, :], woT[mt * P:(mt + 1) * P, :])

            # ================= PHASE A: projections + rope/norm ==========
            # proj(sc) emits only PE matmuls; the drains/rope of chunk sc
            # are emitted after proj(sc+1) so DVE/Act work overlaps PE.
            acc_tiles = {}

            def emit_proj(sc):
                q0 = sc * SC
                qps = [ps.tile([P, SC], F32, tag="big", bufs=6,
                               name=f"qps{sc}_{i}") for i in range(G)]
                kps = ps.tile([P, SC], F32, tag="big", bufs=6,
                              name=f"kps{sc}")
                vps = ps.tile([P, SC], F32, tag="big", bufs=6,
                              name=f"vps{sc}")
                for ht in range(NHT):
                    xt = xw.tile([P, SC], BF16, tag="xt",
                                 name=f"xt{sc}_{ht}")
                    nc.sync.dma_start(xt, xT[ht * P:(ht + 1) * P, q0:q0 + SC])
                    st = ht == 0
                    sp = ht == NHT - 1
                    for mt in range(G):
                        nc.tensor.matmul(
                            qps[mt], wq_sb[:, ht, mt * P:(mt + 1) * P], xt,
                            start=st, stop=sp,
                        )
                    nc.tensor.matmul(kps, wk_sb[:, ht, :], xt,
                                     start=st, stop=sp)
                    nc.tensor.matmul(vps, wv_sb[:, ht, :], xt,
                                     start=st, stop=sp)
                acc_tiles[sc] = (qps, kps, vps)

            def emit_drains(sc):
                q0 = sc * SC
                qps, kps, vps = acc_tiles.pop(sc)
                # -- Q heads first: frees the "big" PSUM slots in the same
                # order the next chunk's projections re-allocate them.
                for h in range(G):
                    qraw = scr.tile([P, SC], BF16, tag="raw", bufs=3,
                                    name=f"qraw{sc}_{h}")
                    nc.vector.tensor_copy(qraw, qps[h])
                    sqq = scr.tile([P, SC], BF16, tag="sq2",
                                   name=f"sqq{sc}_{h}")
                    nc.vector.tensor_mul(sqq, qraw, qraw)
                    ssbq = ps.tile([P, SC], F32, tag="rot", bufs=2,
                                   name=f"ssbq{sc}_{h}")
                    nc.tensor.matmul(ssbq, ones_bf, sqq, start=True, stop=True)
                    sqs = scr.tile([P, SC], F32, tag="sqs",
                                   name=f"sqs{sc}_{h}")
                    nc.scalar.activation(sqs, ssbq, Sqrt, bias=bias_qeps,
                                         scale=1.0 / P)
                    rqf = scr.tile([P, SC], F32, tag="rqf",
                                   name=f"rqf{sc}_{h}")
                    nc.vector.reciprocal_approx_fast(rqf, sqs)
                    qrot = scr.tile([P, SC], BF16, tag="rot2",
                                    name=f"qrot{sc}_{h}")
                    nc.sync.dma_start(qrot[0:64], qraw[64:128])
                    nc.sync.dma_start(qrot[64:128], qraw[0:64])
                    t1 = scr.tile([P, SC], BF16, tag="t1",
                                  name=f"qt1{sc}_{h}")
                    nc.vector.tensor_mul(t1, qrot, sq_sb[:, q0:q0 + SC])
                    qpre = scr.tile([P, SC], BF16, tag="pre",
                                    name=f"qpre{sc}_{h}")
                    nc.vector.tensor_mul(qpre, qraw, cq_sb[:, q0:q0 + SC])
                    nc.vector.tensor_add(qpre, qpre, t1)
                    nc.vector.tensor_mul(qr_all[:, h, q0:q0 + SC], qpre, rqf)

                # -- K: rope into KR; norm factor in transposed layout via
                # tiny matmuls (becomes the Exp scale in phase B).
                kraw = scr.tile([P, SC], BF16, tag="raw", bufs=3,
                                name=f"kraw{sc}")
                nc.vector.tensor_copy(kraw, kps)
                sqk = scr.tile([P, SC], BF16, tag="sq2", name=f"sqk{sc}")
                nc.vector.tensor_mul(sqk, kraw, kraw)
                rkf_s = scr.tile([P, 4], F32, tag="rkf_s", name=f"rkfs{sc}")
                for j in range(4):
                    ssqT = ps.tile([P, 1], F32, tag="rot", bufs=2,
                                   name=f"ssqT{sc}_{j}")
                    nc.tensor.matmul(ssqT, sqk[:, j * P:(j + 1) * P],
                                     ones_bf[:, 0:1], start=True, stop=True)
                    nc.scalar.activation(rkf_s[:, j:j + 1], ssqT, Sqrt,
                                         bias=bias_keps, scale=1.0)
                nc.vector.reciprocal(rkfT[:, sc * 4:(sc + 1) * 4], rkf_s)
                krot = scr.tile([P, SC], BF16, tag="rot2", name=f"krot{sc}")
                nc.sync.dma_start(krot[0:64], kraw[64:128])
                nc.sync.dma_start(krot[64:128], kraw[0:64])
                kt1 = scr.tile([P, SC], BF16, tag="t1", name=f"kt1{sc}")
                nc.vector.tensor_mul(kt1, krot, sk_sb[:, q0:q0 + SC])
                kpre = scr.tile([P, SC], BF16, tag="pre", name=f"kpre{sc}")
                nc.vector.tensor_mul(kpre, kraw, ck_sb[:, q0:q0 + SC])
                nc.vector.tensor_add(KR[:, q0:q0 + SC], kpre, kt1)

                # -- V: cast + transpose [d, s] -> [s, d] tiles
                vtmp = scr.tile([P, SC], BF16, tag="raw", bufs=3,
                                name=f"vtmp{sc}")
                nc.vector.tensor_copy(vtmp, vps)
                for j in range(SC // P):
                    tp = ps.tile([P, P], BF16, tag="rot", bufs=2,
                                 name=f"tp{sc}_{j}")
                    nc.tensor.transpose(tp, vtmp[:, j * P:(j + 1) * P],
                                        identity)
                    nc.vector.tensor_copy(Vs[:, sc * 4 + j, :], tp)

            emit_proj(0)
            for sc in range(1, NSC):
                emit_proj(sc)
                emit_drains(sc - 1)
            emit_drains(NSC - 1)

            # ================= PHASE B: attention + out-projection ========
            for sc in range(NSC):
                q0 = sc * SC
                ots = []
                for h in range(G):
                    dnp = ps.tile([P, SC], F32, tag="big", bufs=6,
                                  name=f"dnp{sc}_{h}")
                    avp = ps.tile([P, SC], F32, tag="big", bufs=6,
                                  name=f"avp{sc}_{h}")
                    nkt = (sc + 1) * 4
                    for kt in range(nkt):
                        ptps = ps.tile([P, SC], F32, tag="rot", bufs=2,
                                       name=f"ptps{sc}_{h}_{kt}")
                        nc.tensor.matmul(
                            ptps, KR[:, kt * P:(kt + 1) * P],
                            qr_all[:, h, q0:q0 + SC],
                            start=True, stop=True,
                        )
                        pt = ptp.tile([P, SC], BF16, tag="pt",
                                      name=f"pt{sc}_{h}_{kt}")
                        nc.scalar.activation(pt, ptps, Exp, bias=0.0,
                                             scale=rkfT[:, kt:kt + 1])
                        if kt >= sc * 4:
                            nc.vector.tensor_mul(pt, pt, masks[kt - sc * 4])
                        nc.tensor.matmul(dnp, ones_bf, pt,
                                         start=(kt == 0), stop=(kt == nkt - 1))
                        nc.tensor.matmul(avp, Vs[:, kt, :], pt,
                                         start=(kt == 0), stop=(kt == nkt - 1))
                    rcp = scr.tile([P, SC], F32, tag="rcp",
                                   name=f"rcp{sc}_{h}")
                    nc.vector.reciprocal_approx_fast(rcp, dnp)
                    ot = otp.tile([P, SC], BF16, tag="ot",
                                  name=f"ot{sc}_{h}")
                    nc.vector.tensor_mul(ot, avp, rcp)
                    ots.append(ot)

                # ---- output projection for this q chunk ----
                for stl in range(SC // P):
                    srow = q0 + stl * P
                    for hc in range(HID // SC):
                        yps = ps.tile([P, SC], F32, tag="rot", bufs=2,
                                      name=f"yps{sc}_{stl}_{hc}")
                        for h in range(G):
                            nc.tensor.matmul(
                                yps, ots[h][:, stl * P:(stl + 1) * P],
                                wo_sb[:, h, hc * SC:(hc + 1) * SC],
                                start=(h == 0), stop=(h == G - 1),
                            )
                        ys = ysp.tile([P, SC], BF16, tag="ys",
                                      name=f"ys{sc}_{stl}_{hc}")
                        if hc % 2 == 0:
                            nc.scalar.copy(ys, yps)
                        else:
                            nc.vector.tensor_copy(ys, yps)
                        nc.sync.dma_start(
                            y[srow:srow + P, hc * SC:(hc + 1) * SC], ys)

    nc.finalize()
    return nc


def shard_inputs(x, wq, wk, wv, wo, q_norm_w, k_norm_w, cos_table, sin_table,
                 positions, **_ignored):
    """Host-side sharding: returns the list of 8 per-core input maps."""
    import ml_dtypes
    bf16 = ml_dtypes.bfloat16

    x = np.asarray(x, np.float32)
    pos = np.asarray(positions).astype(np.int64)
    cos_sel = np.asarray(cos_table, np.float32)[pos]   # [S, D]
    sin_sel = np.asarray(sin_table, np.float32)[pos]
    qw = np.asarray(q_norm_w, np.float32)
    kw = np.asarray(k_norm_w, np.float32)
    # fold norm weights into the transposed rope tables:
    # w * rope(q') == q'*(w*cos) + rot(q')*(w*sin)
    # also fold rotate-half's minus sign into sin rows 0..63:
    # rope(z) = z*cos + [-z2; z1]*sin = z*cos + [z2; z1]*sin_eff
    sign = np.ones((1, D), np.float32)
    sign[0, :D // 2] = -1.0
    cosq = np.ascontiguousarray((cos_sel * qw).T).astype(bf16)     # [D, S]
    sinq = np.ascontiguousarray((sin_sel * qw * sign).T).astype(bf16)
    cosk = np.ascontiguousarray((cos_sel * kw).T).astype(bf16)
    sink = np.ascontiguousarray((sin_sel * kw * sign).T).astype(bf16)
    xTf = np.ascontiguousarray(x.reshape(S, HID).T).astype(bf16)   # [HID, S]
    wq = np.asarray(wq, np.float32)
    wk = np.asarray(wk, np.float32)
    wv = np.asarray(wv, np.float32)
    wo = np.asarray(wo, np.float32)

    in_maps = []
    for c in range(N_CORES):
        m = {
            "xT": xTf,
            "wqT": np.ascontiguousarray(
                wq[c * G * P:(c + 1) * G * P, :].T).astype(bf16),
            "wkT": np.ascontiguousarray(
                wk[c * P:(c + 1) * P, :].T).astype(bf16),
            "wvT": np.ascontiguousarray(
                wv[c * P:(c + 1) * P, :].T).astype(bf16),
            "woT": np.ascontiguousarray(
                wo[:, c * G * P:(c + 1) * G * P].T).astype(bf16),
            "cosq": cosq, "sinq": sinq, "cosk": cosk, "sink": sink,
        }
        in_maps.append(m)
    return in_maps


_NC = None


def _get_nc():
    global _NC
    if _NC is None:
        _NC = build_program()
    return _NC


def run_on_device(in_maps, trace=False, **kw):
    from concourse.bass_utils import run_bass_kernel_spmd
    nc = _get_nc()
    return run_bass_kernel_spmd(nc, in_maps, list(range(N_CORES)),
                                trace=trace, **kw)


def kernel(**inputs):
    in_maps = shard_inputs(**inputs)
    res = run_on_device(in_maps).results
    y = np.zeros((S, HID), np.float32)
    for c in range(N_CORES):
        y += np.asarray(res[c]["y"], np.float32)
    return y.reshape(1, S, HID)


# revision 4
# speedup vs baseline: 1.7286x; 1.1759x over previous
"""GQA attention prefill kernel for Trainium2 (Bass/Tile), 8-way tensor
parallel over heads.

Problem (hardcoded): B=1, S=2048, HID=4096, NH=32, KVH=8, D=128, causal
prefill with per-head RMSNorm on q/k and RoPE, positions = arange(S).

Sharding: core c owns kv-head c and q-heads 4c..4c+3. wq/wo sharded on the
head dim, wk/wv on the kv-head dim; x, rope tables replicated. Each core
computes its 4 heads' contribution through wo; the host sums the 8 partial
outputs.

v3 design:
- Full bf16 datapath; matmuls accumulate in f32 PSUM. bf16 moving operands
  run at the same 1 cycle/row PE rate as f32r but halve DMA/SBUF traffic
  and double DVE throughput.
- All weights resident in SBUF, loaded via a handful of large batched DMAs
  ordered so chunk-0 projections stream behind the transfers (v2 lost
  ~80us to 100+ serialized DMA triggers before the first x tile).
- Software pipelining: chunk sc's PSUM drains + rope/rms-norm are emitted
  after chunk sc+1's projection matmuls, so DVE/Act work hides under PE.
- PSUM layout: 2-bank [P,2,SC] "dual" tiles (K+V proj pair, Q-head pairs,
  score-tile pairs) + 1-bank "single" tiles (softmax accumulators, out-proj
  groups). Exp is evaluated once per score-tile PAIR (halves Act-engine
  instruction count; the Act engine gated attention in v2).
- Reciprocals via the ~5x faster reciprocal_approx_fast custom DVE op.
- All output-projection PSUM drains on the Vector engine; one y DMA per
  128-row stripe.
"""

import numpy as np

import concourse.bass as bass
import concourse.mybir as mybir
import concourse.tile as tile
from concourse import bacc

P = 128
S = 2048
HID = 4096
D = 128
G = 4            # q heads per core
NHT = HID // P   # 32 h-tiles (contraction)
SC = 512         # seq chunk
NSC = S // SC    # 4
NKT = S // P     # 16 k-tiles
EPS = 1e-6
N_CORES = 8

F32 = mybir.dt.float32
BF16 = mybir.dt.bfloat16

Sqrt = mybir.ActivationFunctionType.Sqrt
Exp = mybir.ActivationFunctionType.Exp


def build_program():
    nc = bacc.Bacc("TRN2", target_bir_lowering=False, debug=False)

    xT = nc.dram_tensor("xT", [HID, S], BF16, kind="ExternalInput").ap()
    wqT = nc.dram_tensor("wqT", [HID, G * P], BF16, kind="ExternalInput").ap()
    wkT = nc.dram_tensor("wkT", [HID, P], BF16, kind="ExternalInput").ap()
    wvT = nc.dram_tensor("wvT", [HID, P], BF16, kind="ExternalInput").ap()
    woT = nc.dram_tensor("woT", [G * P, HID], BF16, kind="ExternalInput").ap()
    cosq = nc.dram_tensor("cosq", [D, S], BF16, kind="ExternalInput").ap()
    sinq = nc.dram_tensor("sinq", [D, S], BF16, kind="ExternalInput").ap()
    cosk = nc.dram_tensor("cosk", [D, S], BF16, kind="ExternalInput").ap()
    sink = nc.dram_tensor("sink", [D, S], BF16, kind="ExternalInput").ap()
    y = nc.dram_tensor("y", [S, HID], BF16, kind="ExternalOutput").ap()

    with tile.TileContext(nc) as tc:
        with (
            tc.tile_pool(name="const", bufs=1) as const,
            tc.tile_pool(name="xw", bufs=3) as xw,
            tc.tile_pool(name="scr", bufs=2) as scr,
            tc.tile_pool(name="ptp", bufs=3) as ptp,
            tc.tile_pool(name="otp", bufs=5) as otp,
            tc.tile_pool(name="ysp", bufs=2) as ysp,
            tc.tile_pool(name="ps", bufs=1, space="PSUM") as ps,
        ):
            # ---- resident weights: batched DMAs, ordered so the chunk-0
            # projection loop can stream right behind them ----
            wk_sb = const.tile([P, NHT, P], BF16)
            wv_sb = const.tile([P, NHT, P], BF16)
            wq_sb = const.tile([P, NHT, G * P], BF16)
            nc.sync.dma_start(
                wk_sb, wkT.rearrange("(ht p) c -> p ht c", p=P))
            nc.sync.dma_start(
                wv_sb, wvT.rearrange("(ht p) c -> p ht c", p=P))
            WQB = 8  # ht-tiles per wq DMA block
            for b in range(NHT // WQB):
                nc.sync.dma_start(
                    wq_sb[:, b * WQB:(b + 1) * WQB, :],
                    wqT[b * WQB * P:(b + 1) * WQB * P, :].rearrange(
                        "(ht p) c -> p ht c", p=P))

            # ---- constants ----
            f32tmp = const.tile([P, SC], F32)
            identity = const.tile([P, P], BF16)
            nc.gpsimd.memset(f32tmp[:, 0:P], 0.0)
            nc.gpsimd.affine_select(
                f32tmp[:, 0:P], f32tmp[:, 0:P],
                compare_op=mybir.AluOpType.not_equal, fill=1.0,
                base=0, pattern=[[-1, P]], channel_multiplier=1,
            )
            nc.vector.tensor_copy(identity, f32tmp[:, 0:P])
            ones_bf = const.tile([P, P], BF16)
            nc.gpsimd.memset(f32tmp[:, 0:P], 1.0)
            nc.vector.tensor_copy(ones_bf, f32tmp[:, 0:P])
            # causal masks for the 4 diagonal k-tiles of a q chunk:
            # keep (1.0) where q_local >= 128*j + k_local
            masks = const.tile([P, 4, SC], BF16)
            for j in range(4):
                nc.gpsimd.memset(f32tmp, 1.0)
                nc.gpsimd.affine_select(
                    f32tmp, f32tmp, pattern=[[1, SC]],
                    compare_op=mybir.AluOpType.is_ge,
                    fill=0.0, base=-P * j, channel_multiplier=-1,
                )
                nc.vector.tensor_copy(masks[:, j, :], f32tmp)

            bias_keps = const.tile([P, 1], F32)
            nc.gpsimd.memset(bias_keps, float(P) * EPS)
            bias_qeps = const.tile([P, 1], F32)
            nc.gpsimd.memset(bias_qeps, EPS)

            # ---- persistent tensors ----
            KR = const.tile([P, S], BF16)        # roped+normed K
            Vs = const.tile([P, NKT, P], BF16)   # V, [s-in-tile, k-tile, d]
            qr_all = const.tile([P, G, S], BF16)  # roped+normed Q per head

            # tables + wo arrive during chunk-0/1 projections
            ck_sb = const.tile([P, S], BF16)
            sk_sb = const.tile([P, S], BF16)
            cq_sb = const.tile([P, S], BF16)
            sq_sb = const.tile([P, S], BF16)
            wo_sb = const.tile([P, G, HID], BF16)

            def emit_tail_loads():
                nc.sync.dma_start(ck_sb, cosk)
                nc.sync.dma_start(sk_sb, sink)
                nc.sync.dma_start(cq_sb, cosq)
                nc.sync.dma_start(sq_sb, sinq)
                for mt in range(G):
                    nc.sync.dma_start(wo_sb[:, mt, :],
                                      woT[mt * P:(mt + 1) * P, :])

            # ================= PHASE A: projections + rope/norm ==========
            acc_tiles = {}
            XB = 4  # ht-tiles per x DMA block

            def emit_proj(sc):
                q0 = sc * SC
                kvps = ps.tile([P, 2, SC], F32, tag="dual", bufs=3,
                               name=f"kvps{sc}")
                qps = [ps.tile([P, 2, SC], F32, tag="dual", bufs=3,
                               name=f"qps{sc}_{i}") for i in range(2)]
                xt = None
                for ht in range(NHT):
                    if ht % XB == 0:
                        xt = xw.tile([P, XB, SC], BF16, tag="xt",
                                     name=f"xt{sc}_{ht}")
                        nc.sync.dma_start(
                            xt, xT[ht * P:(ht + XB) * P, q0:q0 + SC]
                            .rearrange("(b p) c -> p b c", p=P))
                    st = ht == 0
                    sp = ht == NHT - 1
                    xs = xt[:, ht % XB, :]
                    for mt in range(G):
                        nc.tensor.matmul(
                            qps[mt // 2][:, mt % 2, :],
                            wq_sb[:, ht, mt * P:(mt + 1) * P], xs,
                            start=st, stop=sp,
                        )
                    nc.tensor.matmul(kvps[:, 0, :], wk_sb[:, ht, :], xs,
                                     start=st, stop=sp)
                    nc.tensor.matmul(kvps[:, 1, :], wv_sb[:, ht, :], xs,
                                     start=st, stop=sp)
                acc_tiles[sc] = (kvps, qps)

            def emit_drains(sc):
                q0 = sc * SC
                kvps, qps = acc_tiles.pop(sc)
                # K/V casts first: frees the kv dual slot the next chunk's
                # projections re-allocate first.
                kraw = scr.tile([P, SC], BF16, tag="kraw", name=f"kraw{sc}")
                nc.vector.tensor_copy(kraw, kvps[:, 0, :])
                vtmp = scr.tile([P, SC], BF16, tag="vtmp", name=f"vtmp{sc}")
                nc.vector.tensor_copy(vtmp, kvps[:, 1, :])
                qraw = scr.tile([P, G, SC], BF16, tag="qraw",
                                name=f"qraw{sc}")
                for h in range(G):
                    nc.vector.tensor_copy(qraw[:, h, :],
                                          qps[h // 2][:, h % 2, :])

                # -- K: rms-norm factor (carries softmax 1/sqrt(d)) + rope
                sqk = scr.tile([P, SC], BF16, tag="sq2", name=f"sqk{sc}")
                nc.vector.tensor_mul(sqk, kraw, kraw)
                ssb = ps.tile([P, SC], F32, tag="single", bufs=2,
                              name=f"ssbk{sc}")
                nc.tensor.matmul(ssb, ones_bf, sqk, start=True, stop=True)
                sqs = scr.tile([P, SC], F32, tag="sqs", bufs=1,
                               name=f"sqsk{sc}")
                nc.scalar.activation(sqs, ssb, Sqrt, bias=bias_keps,
                                     scale=1.0)
                rkf = scr.tile([P, SC], F32, tag="rqf", name=f"rkf{sc}")
                nc.vector.reciprocal_approx_fast(rkf, sqs)
                krot = scr.tile([P, SC], BF16, tag="krot", name=f"krot{sc}")
                nc.sync.dma_start(krot[0:64], kraw[64:128])
                nc.sync.dma_start(krot[64:128], kraw[0:64])
                kt1 = scr.tile([P, SC], BF16, tag="t1", name=f"kt1{sc}")
                nc.vector.tensor_mul(kt1, krot, sk_sb[:, q0:q0 + SC])
                kpre = scr.tile([P, SC], BF16, tag="pre", name=f"kpre{sc}")
                nc.vector.tensor_mul(kpre, kraw, ck_sb[:, q0:q0 + SC])
                nc.vector.tensor_add(kpre, kpre, kt1)
                nc.vector.tensor_mul(KR[:, q0:q0 + SC], kpre, rkf)

                # -- V: transpose [d, s] -> [s, d] tiles
                for j in range(SC // P):
                    tp = ps.tile([P, P], BF16, tag="single", bufs=2,
                                 name=f"tp{sc}_{j}")
                    nc.tensor.transpose(tp, vtmp[:, j * P:(j + 1) * P],
                                        identity)
                    nc.vector.tensor_copy(Vs[:, sc * 4 + j, :], tp)

                # -- Q heads: rms-norm + rope
                qrot = scr.tile([P, G, SC], BF16, tag="qrot",
                                name=f"qrot{sc}")
                nc.sync.dma_start(qrot[0:64], qraw[64:128])
                nc.sync.dma_start(qrot[64:128], qraw[0:64])
                for h in range(G):
                    sqq = scr.tile([P, SC], BF16, tag="sq2",
                                   name=f"sqq{sc}_{h}")
                    nc.vector.tensor_mul(sqq, qraw[:, h, :], qraw[:, h, :])
                    ssbq = ps.tile([P, SC], F32, tag="single", bufs=2,
                                   name=f"ssbq{sc}_{h}")
                    nc.tensor.matmul(ssbq, ones_bf, sqq, start=True,
                                     stop=True)
                    sqs_q = scr.tile([P, SC], F32, tag="sqs", bufs=1,
                                     name=f"sqsq{sc}_{h}")
                    nc.scalar.activation(sqs_q, ssbq, Sqrt, bias=bias_qeps,
                                         scale=1.0 / P)
                    rqf = scr.tile([P, SC], F32, tag="rqf",
                                   name=f"rqf{sc}_{h}")
                    nc.vector.reciprocal_approx_fast(rqf, sqs_q)
                    t1 = scr.tile([P, SC], BF16, tag="t1",
                                  name=f"qt1{sc}_{h}")
                    nc.vector.tensor_mul(t1, qrot[:, h, :],
                                         sq_sb[:, q0:q0 + SC])
                    qpre = scr.tile([P, SC], BF16, tag="pre",
                                    name=f"qpre{sc}_{h}")
                    nc.vector.tensor_mul(qpre, qraw[:, h, :],
                                         cq_sb[:, q0:q0 + SC])
                    nc.vector.tensor_add(qpre, qpre, t1)
                    nc.vector.tensor_mul(qr_all[:, h, q0:q0 + SC], qpre, rqf)

            emit_proj(0)
            emit_tail_loads()
            for sc in range(1, NSC):
                emit_proj(sc)
                emit_drains(sc - 1)
            emit_drains(NSC - 1)

            # ================= PHASE B: attention + out-projection ========
            for sc in range(NSC):
                q0 = sc * SC
                ots = []
                for h in range(G):
                    dnp = ps.tile([P, SC], F32, tag="single", bufs=2,
                                  name=f"dnp{sc}_{h}")
                    avp = ps.tile([P, SC], F32, tag="single", bufs=2,
                                  name=f"avp{sc}_{h}")
                    nkt = (sc + 1) * 4
                    for kp in range(nkt // 2):
                        ptps = ps.tile([P, 2, SC], F32, tag="dual", bufs=3,
                                       name=f"ptps{sc}_{h}_{kp}")
                        for j in range(2):
                            kt = 2 * kp + j
                            nc.tensor.matmul(
                                ptps[:, j, :], KR[:, kt * P:(kt + 1) * P],
                                qr_all[:, h, q0:q0 + SC],
                                start=True, stop=True,
                            )
                        pt = ptp.tile([P, 2, SC], BF16, tag="pt",
                                      name=f"pt{sc}_{h}_{kp}")
                        nc.scalar.activation(pt, ptps, Exp)
                        if 2 * kp >= sc * 4:
                            jm = 2 * kp - sc * 4
                            nc.vector.tensor_mul(pt, pt,
                                                 masks[:, jm:jm + 2, :])
                        for j in range(2):
                            kt = 2 * kp + j
                            nc.tensor.matmul(
                                dnp, ones_bf, pt[:, j, :],
                                start=(kt == 0), stop=(kt == nkt - 1))
                            nc.tensor.matmul(
                                avp, Vs[:, kt, :], pt[:, j, :],
                                start=(kt == 0), stop=(kt == nkt - 1))
                    rcp = scr.tile([P, SC], F32, tag="rcp",
                                   name=f"rcp{sc}_{h}")
                    nc.vector.reciprocal_approx_fast(rcp, dnp)
                    ot = otp.tile([P, SC], BF16, tag="ot",
                                  name=f"ot{sc}_{h}")
                    nc.vector.tensor_mul(ot, avp, rcp)
                    ots.append(ot)

                # ---- output projection for this q chunk ----
                for stl in range(SC // P):
                    srow = q0 + stl * P
                    ys = ysp.tile([P, HID], BF16, tag="ys",
                                  name=f"ys{sc}_{stl}")
                    for hc in range(HID // SC):
                        yps = ps.tile([P, SC], F32, tag="single", bufs=2,
                                      name=f"yps{sc}_{stl}_{hc}")
                        for h in range(G):
                            nc.tensor.matmul(
                                yps, ots[h][:, stl * P:(stl + 1) * P],
                                wo_sb[:, h, hc * SC:(hc + 1) * SC],
                                start=(h == 0), stop=(h == G - 1),
                            )
                        nc.vector.tensor_copy(
                            ys[:, hc * SC:(hc + 1) * SC], yps)
                    nc.sync.dma_start(y[srow:srow + P, :], ys)

    nc.finalize()
    return nc


def shard_inputs(x, wq, wk, wv, wo, q_norm_w, k_norm_w, cos_table, sin_table,
                 positions, **_ignored):
    """Host-side sharding: returns the list of 8 per-core input maps."""
    import ml_dtypes
    bf16 = ml_dtypes.bfloat16

    x = np.asarray(x, np.float32)
    pos = np.asarray(positions).astype(np.int64)
    cos_sel = np.asarray(cos_table, np.float32)[pos]   # [S, D]
    sin_sel = np.asarray(sin_table, np.float32)[pos]
    qw = np.asarray(q_norm_w, np.float32)
    kw = np.asarray(k_norm_w, np.float32)
    # fold norm weights into the transposed rope tables:
    # w * rope(q') == q'*(w*cos) + rot(q')*(w*sin)
    # also fold rotate-half's minus sign into sin rows 0..63:
    # rope(z) = z*cos + [-z2; z1]*sin = z*cos + [z2; z1]*sin_eff
    sign = np.ones((1, D), np.float32)
    sign[0, :D // 2] = -1.0
    cosq = np.ascontiguousarray((cos_sel * qw).T).astype(bf16)     # [D, S]
    sinq = np.ascontiguousarray((sin_sel * qw * sign).T).astype(bf16)
    cosk = np.ascontiguousarray((cos_sel * kw).T).astype(bf16)
    sink = np.ascontiguousarray((sin_sel * kw * sign).T).astype(bf16)
    xTf = np.ascontiguousarray(x.reshape(S, HID).T).astype(bf16)   # [HID, S]
    wq = np.asarray(wq, np.float32)
    wk = np.asarray(wk, np.float32)
    wv = np.asarray(wv, np.float32)
    wo = np.asarray(wo, np.float32)

    in_maps = []
    for c in range(N_CORES):
        m = {
            "xT": xTf,
            "wqT": np.ascontiguousarray(
                wq[c * G * P:(c + 1) * G * P, :].T).astype(bf16),
            "wkT": np.ascontiguousarray(
                wk[c * P:(c + 1) * P, :].T).astype(bf16),
            "wvT": np.ascontiguousarray(
                wv[c * P:(c + 1) * P, :].T).astype(bf16),
            "woT": np.ascontiguousarray(
                wo[:, c * G * P:(c + 1) * G * P].T).astype(bf16),
            "cosq": cosq, "sinq": sinq, "cosk": cosk, "sink": sink,
        }
        in_maps.append(m)
    return in_maps


_NC = None


def _get_nc():
    global _NC
    if _NC is None:
        _NC = build_program()
    return _NC


def run_on_device(in_maps, trace=False, **kw):
    from concourse.bass_utils import run_bass_kernel_spmd
    nc = _get_nc()
    return run_bass_kernel_spmd(nc, in_maps, list(range(N_CORES)),
                                trace=trace, **kw)


def kernel(**inputs):
    in_maps = shard_inputs(**inputs)
    res = run_on_device(in_maps).results
    y = np.zeros((S, HID), np.float32)
    for c in range(N_CORES):
        y += np.asarray(res[c]["y"], np.float32)
    return y.reshape(1, S, HID)
